# revision 27
# baseline (speedup 1.0000x reference)
"""DirectedHebbianGraph fused Bass kernel for 8 trn2 NeuronCores.

Sharding: data-parallel over batch (B=4096 -> 512 rows/core). Combined
forward weights A_i = w_i + W*heb_i are folded on the host, split into
fp16 (primary, residual) pairs, and replicated. Each fp32 matmul runs as
three 1-cycle/row fp16 terms: X@Y ~= X1@Y1 + X2@Y1 + X1@Y2, where
X = X1 + X2 captures 22+ significand bits (fp16 subnormals are exact on
the PE), giving fp32-grade accuracy at ~3x fp32 matmul speed. Per-shard
Hebbian partial sums U = lhs.T @ preact go back in fp32; the host sums
the 8 partials (the sharding hint's all-reduce, done at unshard time),
applies the clamp, and computes the scalar outputs.
"""

from contextlib import ExitStack

import numpy as np

import concourse.bacc as bacc
import concourse.mybir as mybir
import concourse.tile as tile
from concourse import bass_utils
from concourse.masks import make_identity

B, D, H, O = 4096, 1024, 1024, 1024
NCORES = 8
BS = B // NCORES      # 512 rows per core
MB = BS // 128        # 4 batch chunks
KD = D // 128         # 8 contraction chunks
NH = H // 512         # 2 free-dim halves
ALPHA = np.float32(0.3)
CLAMP = np.float32(0.3)
F32 = mybir.dt.float32
F16 = mybir.dt.float16
ADD = mybir.AluOpType.add
SUB = mybir.AluOpType.subtract

_CACHE = {}


def _build():
    nc = bacc.Bacc("TRN2", target_bir_lowering=False, debug=False, num_devices=NCORES)

    xT1 = nc.dram_tensor("xT1", [D, BS], F16, kind="ExternalInput")
    xT2 = nc.dram_tensor("xT2", [D, BS], F16, kind="ExternalInput")
    xn1 = nc.dram_tensor("xn1", [BS, D], F16, kind="ExternalInput")
    xn2 = nc.dram_tensor("xn2", [BS, D], F16, kind="ExternalInput")
    A1 = [nc.dram_tensor(f"A{i}p", [D, H], F16, kind="ExternalInput") for i in range(6)]
    A2 = [nc.dram_tensor(f"A{i}r", [D, H], F16, kind="ExternalInput") for i in range(6)]
    y_d = nc.dram_tensor("y", [BS, O], F32, kind="ExternalOutput")
    U = [nc.dram_tensor(f"U{i}", [D, H], F32, kind="ExternalOutput") for i in range(6)]

    with tile.TileContext(nc) as tc, ExitStack() as ctx:
        sb = ctx.enter_context(tc.tile_pool(name="sb", bufs=1))
        wp = ctx.enter_context(tc.tile_pool(name="wp", bufs=6))
        hx = ctx.enter_context(tc.tile_pool(name="hx", bufs=3))
        uev = ctx.enter_context(tc.tile_pool(name="uev", bufs=6))
        scr = ctx.enter_context(tc.tile_pool(name="scr", bufs=4))
        ps = ctx.enter_context(tc.tile_pool(name="ps", bufs=4, space="PSUM"))
        psu = ctx.enter_context(tc.tile_pool(name="psu", bufs=2, space="PSUM"))
        pst = ctx.enter_context(tc.tile_pool(name="pst", bufs=2, space="PSUM"))

        id16 = sb.tile([128, 128], F16, name="id16", tag="id16")
        make_identity(nc, id16[:])

        def load_split(d1, d2, n_tiles, width, prefix):
            t1, t2 = [], []
            for k in range(n_tiles):
                a = sb.tile([128, width], F16, name=f"{prefix}1_{k}", tag=f"{prefix}1_{k}")
                b = sb.tile([128, width], F16, name=f"{prefix}2_{k}", tag=f"{prefix}2_{k}")
                nc.sync.dma_start(a[:], d1.ap()[k * 128 : (k + 1) * 128, :])
                nc.sync.dma_start(b[:], d2.ap()[k * 128 : (k + 1) * 128, :])
                t1.append(a)
                t2.append(b)
            return t1, t2

        def load_w(li, n, k, suffix="", eng=None):
            eng = eng or nc.sync
            w1 = wp.tile([128, 512], F16, name=f"w1_{li}_{n}_{k}{suffix}", tag="w1")
            w2 = wp.tile([128, 512], F16, name=f"w2_{li}_{n}_{k}{suffix}", tag="w2")
            eng.dma_start(w1[:], A1[li].ap()[k * 128 : (k + 1) * 128, n * 512 : (n + 1) * 512])
            eng.dma_start(w2[:], A2[li].ap()[k * 128 : (k + 1) * 128, n * 512 : (n + 1) * 512])
            return w1, w2

        # first weight pairs prefetched ahead of the 16-tile xT bulk load so
        # the first matmul group isn't queued behind it (measured 11.7 us
        # startup PE stall otherwise)
        w_pre0 = {(0, k): load_w(0, 0, k, "p", eng=nc.scalar) for k in range(2)}
        xT_1, xT_2 = load_split(xT1, xT2, KD, BS, "xT")

        def mm3(p, l1, l2, r1, r2, first, last):
            nc.tensor.matmul(p, l1, r1, start=first, stop=False)
            nc.tensor.matmul(p, l2, r1, start=False, stop=False)
            nc.tensor.matmul(p, l1, r2, start=False, stop=last)

        def mm4(p, l1, l2, r1, r2, first, last):
            # 4-term variant: the dropped X2@Y2 cross term matters when the
            # downstream Hebbian reduction multiplies by a nonneg-mean
            # activation (no cancellation of fixed perturbations).
            nc.tensor.matmul(p, l1, r1, start=first, stop=False)
            nc.tensor.matmul(p, l2, r1, start=False, stop=False)
            nc.tensor.matmul(p, l1, r2, start=False, stop=False)
            nc.tensor.matmul(p, l2, r2, start=False, stop=last)

        def forward(li, L1, L2, evict, mm=None, w_pre=None):
            """evict(m, n, psum[128,512]) consumes each forward output tile."""
            mm = mm or mm3
            for n in range(NH):
                pt = [ps.tile([128, 512], F32, name=f"fw{li}_{m}_{n}", tag="ps") for m in range(MB)]
                for k in range(KD):
                    if w_pre and (n, k) in w_pre:
                        w1, w2 = w_pre[(n, k)]
                    else:
                        w1, w2 = load_w(li, n, k)
                    for m in range(MB):
                        msl = slice(m * 128, (m + 1) * 128)
                        mm(
                            pt[m][:],
                            L1[k][:, msl], L2[k][:, msl],
                            w1[:], w2[:],
                            first=(k == 0), last=(k == KD - 1),
                        )
                for m in range(MB):
                    evict(m, n, pt[m])

        def umatmul(ui, L1, L2, R1, R2, U_dram):
            """U partial = L.T @ R, contraction over the 512 batch rows."""
            for dm in range(KD):
                dsl = slice(dm * 128, (dm + 1) * 128)
                for n in range(NH):
                    nsl = slice(n * 512, (n + 1) * 512)
                    p = psu.tile([128, 512], F32, name=f"u{ui}_{dm}_{n}", tag="psu")
                    for k in range(MB):
                        mm3(
                            p[:],
                            L1[k][:, dsl], L2[k][:, dsl],
                            R1[k][:, nsl], R2[k][:, nsl],
                            first=(k == 0), last=(k == MB - 1),
                        )
                    t = uev.tile([128, 512], F32, name=f"uev{ui}_{dm}_{n}", tag="uev")
                    nc.vector.tensor_copy(t[:], p[:])
                    nc.sync.dma_start(U_dram.ap()[dsl, nsl], t[:])

        def transpose_forms(nat1, nat2, name):
            """Transpose fp16 form tiles [MB][128,H] -> [KD][128,BS]."""
            t1 = [sb.tile([128, BS], F16, name=f"{name}1_{d}", tag=f"{name}1_{d}") for d in range(KD)]
            t2 = [sb.tile([128, BS], F16, name=f"{name}2_{d}", tag=f"{name}2_{d}") for d in range(KD)]
            for m in range(MB):
                msl = slice(m * 128, (m + 1) * 128)
                for d in range(KD):
                    dsl = slice(d * 128, (d + 1) * 128)
                    p1 = pst.tile([128, 128], F16, name=f"tp1_{name}_{m}_{d}", tag="tp")
                    nc.tensor.transpose(p1[:], nat1[m][:, dsl], id16[:])
                    nc.vector.tensor_scalar_mul(t1[d][:, msl], p1[:], 16.0)
                    p2 = pst.tile([128, 128], F16, name=f"tp2_{name}_{m}_{d}", tag="tp")
                    nc.tensor.transpose(p2[:], nat2[m][:, dsl], id16[:])
                    nc.vector.tensor_scalar_mul(t2[d][:, msl], p2[:], 16.0)
            return t1, t2

        def hx_group(name):
            g1 = [hx.tile([128, H], F16, name=f"{name}1_{m}", tag=f"hx1_{m}") for m in range(MB)]
            g2 = [hx.tile([128, H], F16, name=f"{name}2_{m}", tag=f"hx2_{m}") for m in range(MB)]
            return g1, g2

        # ---- layer 1: h0 = relu(x @ A0) ----
        h0x1, h0x2 = hx_group("h0x")
        h0_1 = [sb.tile([128, H], F16, name=f"h0n1_{m}", tag=f"h0n1_{m}") for m in range(MB)]
        h0_2 = [sb.tile([128, H], F16, name=f"h0n2_{m}", tag=f"h0n2_{m}") for m in range(MB)]

        def ev_h0(m, n, p):
            nsl = slice(n * 512, (n + 1) * 512)
            q = scr.tile([128, 512], F32, name=f"q0_{m}_{n}", tag="scr")
            nc.vector.tensor_scalar_mul(q[:], p[:], 0.03125)
            nc.vector.tensor_copy(h0x1[m][:, nsl], q[:])
            nc.vector.tensor_tensor(h0x2[m][:, nsl], q[:], h0x1[m][:, nsl], SUB)
            # h0 kept pre-scaled by 1/16 so the U partial sums accumulate at
            # 16x smaller magnitude (less fp32 PSUM ulp noise); the host
            # multiplies U2/U4 back, and transpose eviction un-scales for
            # the forward path. Power of two, so exact.
            r = scr.tile([128, 512], F32, name=f"r0_{m}_{n}", tag="scr")
            nc.vector.tensor_scalar(r[:], q[:], 0.0, 0.0625, mybir.AluOpType.max, mybir.AluOpType.mult)
            nc.vector.tensor_copy(h0_1[m][:, nsl], r[:])
            nc.vector.tensor_tensor(h0_2[m][:, nsl], r[:], h0_1[m][:, nsl], SUB)

        forward(0, xT_1, xT_2, ev_h0, w_pre=w_pre0)
        xn_1, xn_2 = load_split(xn1, xn2, MB, D, "xn")
        umatmul(0, xn_1, xn_2, h0x1, h0x2, U[0])
        h0T_1, h0T_2 = transpose_forms(h0_1, h0_2, "h0T")

        # ---- layer 2: h1 = relu(x @ A1 + h0 @ A2) ----
        h1x1, h1x2 = hx_group("h1x")

        def ev_h1x(m, n, p):
            nsl = slice(n * 512, (n + 1) * 512)
            q = scr.tile([128, 512], F32, name=f"q1_{m}_{n}", tag="scr")
            nc.vector.tensor_scalar_mul(q[:], p[:], 0.03125)
            nc.vector.tensor_copy(h1x1[m][:, nsl], q[:])
            nc.vector.tensor_tensor(h1x2[m][:, nsl], q[:], h1x1[m][:, nsl], SUB)

        forward(1, xT_1, xT_2, ev_h1x)
        umatmul(1, xn_1, xn_2, h1x1, h1x2, U[1])

        h10_1, h10_2 = hx_group("h10")
        h1_1 = [sb.tile([128, H], F16, name=f"h1n1_{m}", tag=f"h1n1_{m}") for m in range(MB)]
        h1_2 = [sb.tile([128, H], F16, name=f"h1n2_{m}", tag=f"h1n2_{m}") for m in range(MB)]

        def ev_h10(m, n, p):
            nsl = slice(n * 512, (n + 1) * 512)
            q = scr.tile([128, 512], F32, name=f"qh_{m}_{n}", tag="scr")
            nc.vector.tensor_scalar_mul(q[:], p[:], 0.03125)
            nc.vector.tensor_copy(h10_1[m][:, nsl], q[:])
            nc.vector.tensor_tensor(h10_2[m][:, nsl], q[:], h10_1[m][:, nsl], SUB)
            r = scr.tile([128, 512], F32, name=f"r1_{m}_{n}", tag="scr")
            nc.vector.tensor_tensor(r[:], q[:], h1x1[m][:, nsl], ADD)
            nc.vector.tensor_tensor(r[:], r[:], h1x2[m][:, nsl], ADD)
            nc.vector.tensor_scalar(r[:], r[:], 0.0, 0.0625, mybir.AluOpType.max, mybir.AluOpType.mult)
            nc.vector.tensor_copy(h1_1[m][:, nsl], r[:])
            nc.vector.tensor_tensor(h1_2[m][:, nsl], r[:], h1_1[m][:, nsl], SUB)

        forward(2, h0T_1, h0T_2, ev_h10, mm=mm4)
        umatmul(2, h0_1, h0_2, h10_1, h10_2, U[2])
        h1T_1, h1T_2 = transpose_forms(h1_1, h1_2, "h1T")

        # ---- layer 3: y = x @ A3 + h0 @ A4 + h1 @ A5 (accum-DMA'd into y) ----
        def ev_y(pref, yt1, yt2):
            def _ev(m, n, p):
                nsl = slice(n * 512, (n + 1) * 512)
                t = uev.tile([128, 512], F32, name=f"yev_{pref}_{m}_{n}", tag="uev")
                nc.vector.tensor_scalar_mul(t[:], p[:], 0.03125)
                nc.vector.tensor_copy(yt1[m][:, nsl], t[:])
                nc.vector.tensor_tensor(yt2[m][:, nsl], t[:], yt1[m][:, nsl], SUB)
                nc.gpsimd.dma_start(
                    y_d.ap()[m * 128 : (m + 1) * 128, n * 512 : (n + 1) * 512],
                    t[:],
                    accum_op=ADD,
                )
            return _ev

        yx1, yx2 = hx_group("yx")
        forward(3, xT_1, xT_2, ev_y("yx", yx1, yx2))
        umatmul(3, xn_1, xn_2, yx1, yx2, U[3])

        y01, y02 = hx_group("y0")
        forward(4, h0T_1, h0T_2, ev_y("y0", y01, y02), mm=mm4)
        umatmul(4, h0_1, h0_2, y01, y02, U[4])

        y11, y12 = hx_group("y1")
        forward(5, h1T_1, h1T_2, ev_y("y1", y11, y12), mm=mm4)
        umatmul(5, h1_1, h1_2, y11, y12, U[5])

    nc.finalize()
    return nc


def _get_nc():
    if "nc" not in _CACHE:
        _CACHE["nc"] = _build()
    return _CACHE["nc"]


def _split(a):
    p = a.astype(np.float16)
    r = (a - p.astype(np.float32)).astype(np.float16)
    return p, r


def run_device(inputs, trace=False):
    x = np.ascontiguousarray(inputs["x"], dtype=np.float32)
    W = np.float32(inputs["W"].reshape(-1)[0])
    w_names = ["x2h0", "x2h1", "h02h1", "x2y", "h02y", "h12y"]
    A_split = [
        _split(
            np.float32(32.0)
            * (inputs[n].astype(np.float32) + W * inputs["heb_" + n].astype(np.float32))
        )
        for n in w_names
    ]
    xT_full = np.ascontiguousarray(x.T)

    in_maps = []
    for c in range(NCORES):
        xTp, xTr = _split(np.ascontiguousarray(xT_full[:, c * BS : (c + 1) * BS]))
        xnp_, xnr = _split(np.ascontiguousarray(x[c * BS : (c + 1) * BS, :]))
        m = {"xT1": xTp, "xT2": xTr, "xn1": xnp_, "xn2": xnr}
        for i in range(6):
            m[f"A{i}p"] = A_split[i][0]
            m[f"A{i}r"] = A_split[i][1]
        in_maps.append(m)

    nc = _get_nc()
    return bass_utils.run_bass_kernel_spmd(
        nc, in_maps, core_ids=list(range(NCORES)), trace=trace
    )


def kernel(**inputs):
    inputs = {k: np.asarray(v) for k, v in inputs.items()}
    res = run_device(inputs)
    results = res.results

    y = np.concatenate([results[c]["y"] for c in range(NCORES)], axis=0)

    w_names = ["x2h0", "x2h1", "h02h1", "x2y", "h02y", "h12y"]
    hebs_new = []
    for i, n in enumerate(w_names):
        U_tot = results[0][f"U{i}"].astype(np.float32).copy()
        for c in range(1, NCORES):
            U_tot += results[c][f"U{i}"]
        if i in (2, 4, 5):
            U_tot *= np.float32(16.0)  # undo the 1/16 pre-scale of h0/h1
        heb_n = np.clip(
            inputs["heb_" + n].astype(np.float32) + ALPHA * U_tot, -CLAMP, CLAMP
        ).astype(np.float32)
        hebs_new.append(heb_n)

    y_out = np.ascontiguousarray(y[:, : O - 2])
    W_new = np.float32(np.tanh(y[:, O - 2]).mean())
    dopa = np.float32(np.tanh(y[:, O - 3]).mean())

    return (y_out, *hebs_new, W_new, dopa)


# revision 29
# speedup vs baseline: 1.0030x; 1.0030x over previous
"""DirectedHebbianGraph fused Bass kernel for 8 trn2 NeuronCores.

Sharding: data-parallel over batch (B=4096 -> 512 rows/core). Combined
forward weights A_i = w_i + W*heb_i are folded on the host, split into
fp16 (primary, residual) pairs, and replicated. Each fp32 matmul runs as
three 1-cycle/row fp16 terms: X@Y ~= X1@Y1 + X2@Y1 + X1@Y2, where
X = X1 + X2 captures 22+ significand bits (fp16 subnormals are exact on
the PE), giving fp32-grade accuracy at ~3x fp32 matmul speed. Per-shard
Hebbian partial sums U = lhs.T @ preact go back in fp32; the host sums
the 8 partials (the sharding hint's all-reduce, done at unshard time),
applies the clamp, and computes the scalar outputs.
"""

from contextlib import ExitStack

import numpy as np

import concourse.bacc as bacc
import concourse.mybir as mybir
import concourse.tile as tile
from concourse import bass_utils
from concourse.masks import make_identity

B, D, H, O = 4096, 1024, 1024, 1024
NCORES = 8
BS = B // NCORES      # 512 rows per core
MB = BS // 128        # 4 batch chunks
KD = D // 128         # 8 contraction chunks
NH = H // 512         # 2 free-dim halves
ALPHA = np.float32(0.3)
CLAMP = np.float32(0.3)
F32 = mybir.dt.float32
F16 = mybir.dt.float16
ADD = mybir.AluOpType.add
SUB = mybir.AluOpType.subtract

_CACHE = {}


def _build():
    nc = bacc.Bacc("TRN2", target_bir_lowering=False, debug=False, num_devices=NCORES)

    xT1 = nc.dram_tensor("xT1", [D, BS], F16, kind="ExternalInput")
    xT2 = nc.dram_tensor("xT2", [D, BS], F16, kind="ExternalInput")
    xn1 = nc.dram_tensor("xn1", [BS, D], F16, kind="ExternalInput")
    xn2 = nc.dram_tensor("xn2", [BS, D], F16, kind="ExternalInput")
    A1 = [nc.dram_tensor(f"A{i}p", [D, H], F16, kind="ExternalInput") for i in range(6)]
    A2 = [nc.dram_tensor(f"A{i}r", [D, H], F16, kind="ExternalInput") for i in range(6)]
    y_d = nc.dram_tensor("y", [BS, O], F32, kind="ExternalOutput")
    U = [nc.dram_tensor(f"U{i}", [D, H], F32, kind="ExternalOutput") for i in range(6)]

    with tile.TileContext(nc) as tc, ExitStack() as ctx:
        sb = ctx.enter_context(tc.tile_pool(name="sb", bufs=1))
        wp = ctx.enter_context(tc.tile_pool(name="wp", bufs=6))
        hx = ctx.enter_context(tc.tile_pool(name="hx", bufs=3))
        uev = ctx.enter_context(tc.tile_pool(name="uev", bufs=6))
        scr = ctx.enter_context(tc.tile_pool(name="scr", bufs=4))
        ps = ctx.enter_context(tc.tile_pool(name="ps", bufs=4, space="PSUM"))
        psu = ctx.enter_context(tc.tile_pool(name="psu", bufs=2, space="PSUM"))
        pst = ctx.enter_context(tc.tile_pool(name="pst", bufs=2, space="PSUM"))

        id16 = sb.tile([128, 128], F16, name="id16", tag="id16")
        make_identity(nc, id16[:])

        def load_split(d1, d2, n_tiles, width, prefix):
            t1, t2 = [], []
            for k in range(n_tiles):
                a = sb.tile([128, width], F16, name=f"{prefix}1_{k}", tag=f"{prefix}1_{k}")
                b = sb.tile([128, width], F16, name=f"{prefix}2_{k}", tag=f"{prefix}2_{k}")
                nc.sync.dma_start(a[:], d1.ap()[k * 128 : (k + 1) * 128, :])
                nc.sync.dma_start(b[:], d2.ap()[k * 128 : (k + 1) * 128, :])
                t1.append(a)
                t2.append(b)
            return t1, t2

        def load_w(li, n, k, suffix="", eng=None):
            eng = eng or nc.sync
            w1 = wp.tile([128, 512], F16, name=f"w1_{li}_{n}_{k}{suffix}", tag="w1")
            w2 = wp.tile([128, 512], F16, name=f"w2_{li}_{n}_{k}{suffix}", tag="w2")
            eng.dma_start(w1[:], A1[li].ap()[k * 128 : (k + 1) * 128, n * 512 : (n + 1) * 512])
            eng.dma_start(w2[:], A2[li].ap()[k * 128 : (k + 1) * 128, n * 512 : (n + 1) * 512])
            return w1, w2

        # first weight pairs prefetched ahead of the 16-tile xT bulk load so
        # the first matmul group isn't queued behind it (measured 11.7 us
        # startup PE stall otherwise)
        w_pre0 = {(0, k): load_w(0, 0, k, "p", eng=nc.scalar) for k in range(2)}
        xT_1, xT_2 = load_split(xT1, xT2, KD, BS, "xT")

        def mm3(p, l1, l2, r1, r2, first, last):
            nc.tensor.matmul(p, l1, r1, start=first, stop=False)
            nc.tensor.matmul(p, l2, r1, start=False, stop=False)
            nc.tensor.matmul(p, l1, r2, start=False, stop=last)

        def mm4(p, l1, l2, r1, r2, first, last):
            # 4-term variant: the dropped X2@Y2 cross term matters when the
            # downstream Hebbian reduction multiplies by a nonneg-mean
            # activation (no cancellation of fixed perturbations).
            nc.tensor.matmul(p, l1, r1, start=first, stop=False)
            nc.tensor.matmul(p, l2, r1, start=False, stop=False)
            nc.tensor.matmul(p, l1, r2, start=False, stop=False)
            nc.tensor.matmul(p, l2, r2, start=False, stop=last)

        def forward(li, L1, L2, evict, mm=None, w_pre=None):
            """evict(m, n, psum[128,512]) consumes each forward output tile."""
            mm = mm or mm3
            for n in range(NH):
                pt = [ps.tile([128, 512], F32, name=f"fw{li}_{m}_{n}", tag="ps") for m in range(MB)]
                for k in range(KD):
                    if w_pre and (n, k) in w_pre:
                        w1, w2 = w_pre[(n, k)]
                    else:
                        w1, w2 = load_w(li, n, k)
                    for m in range(MB):
                        msl = slice(m * 128, (m + 1) * 128)
                        mm(
                            pt[m][:],
                            L1[k][:, msl], L2[k][:, msl],
                            w1[:], w2[:],
                            first=(k == 0), last=(k == KD - 1),
                        )
                for m in range(MB):
                    evict(m, n, pt[m])

        def umatmul(ui, L1, L2, R1, R2, U_dram):
            """U partial = L.T @ R, contraction over the 512 batch rows."""
            for dm in range(KD):
                dsl = slice(dm * 128, (dm + 1) * 128)
                for n in range(NH):
                    nsl = slice(n * 512, (n + 1) * 512)
                    p = psu.tile([128, 512], F32, name=f"u{ui}_{dm}_{n}", tag="psu")
                    for k in range(MB):
                        mm3(
                            p[:],
                            L1[k][:, dsl], L2[k][:, dsl],
                            R1[k][:, nsl], R2[k][:, nsl],
                            first=(k == 0), last=(k == MB - 1),
                        )
                    t = uev.tile([128, 512], F32, name=f"uev{ui}_{dm}_{n}", tag="uev")
                    nc.vector.tensor_copy(t[:], p[:])
                    nc.sync.dma_start(U_dram.ap()[dsl, nsl], t[:])

        def transpose_forms(nat1, nat2, name):
            """Transpose fp16 form tiles [MB][128,H] -> [KD][128,BS]."""
            t1 = [sb.tile([128, BS], F16, name=f"{name}1_{d}", tag=f"{name}1_{d}") for d in range(KD)]
            t2 = [sb.tile([128, BS], F16, name=f"{name}2_{d}", tag=f"{name}2_{d}") for d in range(KD)]
            for m in range(MB):
                msl = slice(m * 128, (m + 1) * 128)
                for d in range(KD):
                    dsl = slice(d * 128, (d + 1) * 128)
                    p1 = pst.tile([128, 128], F16, name=f"tp1_{name}_{m}_{d}", tag="tp")
                    nc.tensor.transpose(p1[:], nat1[m][:, dsl], id16[:])
                    nc.vector.tensor_scalar_mul(t1[d][:, msl], p1[:], 16.0)
                    p2 = pst.tile([128, 128], F16, name=f"tp2_{name}_{m}_{d}", tag="tp")
                    nc.tensor.transpose(p2[:], nat2[m][:, dsl], id16[:])
                    nc.vector.tensor_scalar_mul(t2[d][:, msl], p2[:], 16.0)
            return t1, t2

        def hx_group(name):
            g1 = [hx.tile([128, H], F16, name=f"{name}1_{m}", tag=f"hx1_{m}") for m in range(MB)]
            g2 = [hx.tile([128, H], F16, name=f"{name}2_{m}", tag=f"hx2_{m}") for m in range(MB)]
            return g1, g2

        # ---- layer 1: h0 = relu(x @ A0) ----
        h0x1, h0x2 = hx_group("h0x")
        h0_1 = [sb.tile([128, H], F16, name=f"h0n1_{m}", tag=f"h0n1_{m}") for m in range(MB)]
        h0_2 = [sb.tile([128, H], F16, name=f"h0n2_{m}", tag=f"h0n2_{m}") for m in range(MB)]

        def ev_h0(m, n, p):
            nsl = slice(n * 512, (n + 1) * 512)
            q = scr.tile([128, 512], F32, name=f"q0_{m}_{n}", tag="scr")
            nc.vector.tensor_scalar_mul(q[:], p[:], 0.03125)
            nc.vector.tensor_copy(h0x1[m][:, nsl], q[:])
            nc.vector.tensor_tensor(h0x2[m][:, nsl], q[:], h0x1[m][:, nsl], SUB)
            # h0 kept pre-scaled by 1/16 so the U partial sums accumulate at
            # 16x smaller magnitude (less fp32 PSUM ulp noise); the host
            # multiplies U2/U4 back, and transpose eviction un-scales for
            # the forward path. Power of two, so exact.
            r = scr.tile([128, 512], F32, name=f"r0_{m}_{n}", tag="scr")
            nc.vector.tensor_scalar(r[:], q[:], 0.0, 0.0625, mybir.AluOpType.max, mybir.AluOpType.mult)
            nc.vector.tensor_copy(h0_1[m][:, nsl], r[:])
            nc.vector.tensor_tensor(h0_2[m][:, nsl], r[:], h0_1[m][:, nsl], SUB)

        forward(0, xT_1, xT_2, ev_h0, w_pre=w_pre0)
        xn_1, xn_2 = load_split(xn1, xn2, MB, D, "xn")
        umatmul(0, xn_1, xn_2, h0x1, h0x2, U[0])
        h0T_1, h0T_2 = transpose_forms(h0_1, h0_2, "h0T")

        # ---- layer 2: h1 = relu(x @ A1 + h0 @ A2) ----
        h1x1, h1x2 = hx_group("h1x")

        def ev_h1x(m, n, p):
            nsl = slice(n * 512, (n + 1) * 512)
            q = scr.tile([128, 512], F32, name=f"q1_{m}_{n}", tag="scr")
            nc.vector.tensor_scalar_mul(q[:], p[:], 0.03125)
            nc.vector.tensor_copy(h1x1[m][:, nsl], q[:])
            nc.vector.tensor_tensor(h1x2[m][:, nsl], q[:], h1x1[m][:, nsl], SUB)

        forward(1, xT_1, xT_2, ev_h1x)
        umatmul(1, xn_1, xn_2, h1x1, h1x2, U[1])

        h10_1, h10_2 = hx_group("h10")
        h1_1 = [sb.tile([128, H], F16, name=f"h1n1_{m}", tag=f"h1n1_{m}") for m in range(MB)]
        h1_2 = [sb.tile([128, H], F16, name=f"h1n2_{m}", tag=f"h1n2_{m}") for m in range(MB)]

        def ev_h10(m, n, p):
            nsl = slice(n * 512, (n + 1) * 512)
            q = scr.tile([128, 512], F32, name=f"qh_{m}_{n}", tag="scr")
            nc.vector.tensor_scalar_mul(q[:], p[:], 0.03125)
            nc.vector.tensor_copy(h10_1[m][:, nsl], q[:])
            nc.vector.tensor_tensor(h10_2[m][:, nsl], q[:], h10_1[m][:, nsl], SUB)
            r = scr.tile([128, 512], F32, name=f"r1_{m}_{n}", tag="scr")
            nc.vector.tensor_tensor(r[:], q[:], h1x1[m][:, nsl], ADD)
            nc.vector.tensor_tensor(r[:], r[:], h1x2[m][:, nsl], ADD)
            nc.vector.tensor_scalar(r[:], r[:], 0.0, 0.0625, mybir.AluOpType.max, mybir.AluOpType.mult)
            nc.vector.tensor_copy(h1_1[m][:, nsl], r[:])
            nc.vector.tensor_tensor(h1_2[m][:, nsl], r[:], h1_1[m][:, nsl], SUB)

        forward(2, h0T_1, h0T_2, ev_h10, mm=mm4)
        umatmul(2, h0_1, h0_2, h10_1, h10_2, U[2])
        h1T_1, h1T_2 = transpose_forms(h1_1, h1_2, "h1T")

        # ---- layer 3: y = x @ A3 + h0 @ A4 + h1 @ A5 (accum-DMA'd into y) ----
        def ev_y(pref, yt1, yt2):
            def _ev(m, n, p):
                nsl = slice(n * 512, (n + 1) * 512)
                t = uev.tile([128, 512], F32, name=f"yev_{pref}_{m}_{n}", tag="uev")
                nc.vector.tensor_scalar_mul(t[:], p[:], 0.03125)
                nc.vector.tensor_copy(yt1[m][:, nsl], t[:])
                nc.vector.tensor_tensor(yt2[m][:, nsl], t[:], yt1[m][:, nsl], SUB)
                nc.gpsimd.dma_start(
                    y_d.ap()[m * 128 : (m + 1) * 128, n * 512 : (n + 1) * 512],
                    t[:],
                    accum_op=ADD,
                )
            return _ev

        yx1, yx2 = hx_group("yx")
        forward(3, xT_1, xT_2, ev_y("yx", yx1, yx2))
        umatmul(3, xn_1, xn_2, yx1, yx2, U[3])

        y01, y02 = hx_group("y0")
        forward(4, h0T_1, h0T_2, ev_y("y0", y01, y02), mm=mm4)
        umatmul(4, h0_1, h0_2, y01, y02, U[4])

        y11, y12 = hx_group("y1")
        forward(5, h1T_1, h1T_2, ev_y("y1", y11, y12), mm=mm4)
        umatmul(5, h1_1, h1_2, y11, y12, U[5])

    nc.finalize()
    return nc


def _get_nc():
    if "nc" not in _CACHE:
        _CACHE["nc"] = _build()
    return _CACHE["nc"]


def _split(a):
    p = a.astype(np.float16)
    r = (a - p.astype(np.float32)).astype(np.float16)
    return p, r


def run_device(inputs, trace=False):
    x = np.ascontiguousarray(inputs["x"], dtype=np.float32)
    W = np.float32(inputs["W"].reshape(-1)[0])
    w_names = ["x2h0", "x2h1", "h02h1", "x2y", "h02y", "h12y"]
    A_split = [
        _split(
            np.float32(32.0)
            * (inputs[n].astype(np.float32) + W * inputs["heb_" + n].astype(np.float32))
        )
        for n in w_names
    ]
    xT_full = np.ascontiguousarray(x.T)

    in_maps = []
    for c in range(NCORES):
        xTp, xTr = _split(np.ascontiguousarray(xT_full[:, c * BS : (c + 1) * BS]))
        xnp_, xnr = _split(np.ascontiguousarray(x[c * BS : (c + 1) * BS, :]))
        m = {"xT1": xTp, "xT2": xTr, "xn1": xnp_, "xn2": xnr}
        for i in range(6):
            m[f"A{i}p"] = A_split[i][0]
            m[f"A{i}r"] = A_split[i][1]
        in_maps.append(m)

    nc = _get_nc()
    return bass_utils.run_bass_kernel_spmd(
        nc, in_maps, core_ids=list(range(NCORES)), trace=trace
    )


def kernel(**inputs):
    inputs = {k: np.asarray(v) for k, v in inputs.items()}
    res = run_device(inputs)
    results = res.results

    y = np.concatenate([results[c]["y"] for c in range(NCORES)], axis=0)

    w_names = ["x2h0", "x2h1", "h02h1", "x2y", "h02y", "h12y"]
    hebs_new = []
    for i, n in enumerate(w_names):
        U_tot = results[0][f"U{i}"].astype(np.float32).copy()
        for c in range(1, NCORES):
            U_tot += results[c][f"U{i}"]
        if i in (2, 4, 5):
            U_tot *= np.float32(16.0)  # undo the 1/16 pre-scale of h0/h1
        heb_n = np.clip(
            inputs["heb_" + n].astype(np.float32) + ALPHA * U_tot, -CLAMP, CLAMP
        ).astype(np.float32)
        hebs_new.append(heb_n)

    y_out = np.ascontiguousarray(y[:, : O - 2])
    W_new = np.float32(np.tanh(y[:, O - 2]).mean())
    dopa = np.float32(np.tanh(y[:, O - 3]).mean())

    return (y_out, *hebs_new, W_new, dopa)


# revision 30
# speedup vs baseline: 1.0086x; 1.0056x over previous
"""DirectedHebbianGraph fused Bass kernel for 8 trn2 NeuronCores.

Sharding: data-parallel over batch (B=4096 -> 512 rows/core). Combined
forward weights A_i = w_i + W*heb_i are folded on the host, split into
fp16 (primary, residual) pairs, and replicated. Each fp32 matmul runs as
three 1-cycle/row fp16 terms: X@Y ~= X1@Y1 + X2@Y1 + X1@Y2, where
X = X1 + X2 captures 22+ significand bits (fp16 subnormals are exact on
the PE), giving fp32-grade accuracy at ~3x fp32 matmul speed. Per-shard
Hebbian partial sums U = lhs.T @ preact go back in fp32; the host sums
the 8 partials (the sharding hint's all-reduce, done at unshard time),
applies the clamp, and computes the scalar outputs.
"""

from contextlib import ExitStack

import numpy as np

import concourse.bacc as bacc
import concourse.mybir as mybir
import concourse.tile as tile
from concourse import bass_utils
from concourse.masks import make_identity

B, D, H, O = 4096, 1024, 1024, 1024
NCORES = 8
BS = B // NCORES      # 512 rows per core
MB = BS // 128        # 4 batch chunks
KD = D // 128         # 8 contraction chunks
NH = H // 512         # 2 free-dim halves
ALPHA = np.float32(0.3)
CLAMP = np.float32(0.3)
F32 = mybir.dt.float32
F16 = mybir.dt.float16
ADD = mybir.AluOpType.add
SUB = mybir.AluOpType.subtract

_CACHE = {}


def _build():
    nc = bacc.Bacc("TRN2", target_bir_lowering=False, debug=False, num_devices=NCORES)

    xT1 = nc.dram_tensor("xT1", [D, BS], F16, kind="ExternalInput")
    xT2 = nc.dram_tensor("xT2", [D, BS], F16, kind="ExternalInput")
    xn1 = nc.dram_tensor("xn1", [BS, D], F16, kind="ExternalInput")
    xn2 = nc.dram_tensor("xn2", [BS, D], F16, kind="ExternalInput")
    A1 = [nc.dram_tensor(f"A{i}p", [D, H], F16, kind="ExternalInput") for i in range(6)]
    A2 = [nc.dram_tensor(f"A{i}r", [D, H], F16, kind="ExternalInput") for i in range(6)]
    y_d = nc.dram_tensor("y", [BS, O], F32, kind="ExternalOutput")
    U = [nc.dram_tensor(f"U{i}", [D, H], F32, kind="ExternalOutput") for i in range(6)]

    with tile.TileContext(nc) as tc, ExitStack() as ctx:
        sb = ctx.enter_context(tc.tile_pool(name="sb", bufs=1))
        wp = ctx.enter_context(tc.tile_pool(name="wp", bufs=6))
        hx = ctx.enter_context(tc.tile_pool(name="hx", bufs=3))
        uev = ctx.enter_context(tc.tile_pool(name="uev", bufs=6))
        scr = ctx.enter_context(tc.tile_pool(name="scr", bufs=4))
        ps = ctx.enter_context(tc.tile_pool(name="ps", bufs=4, space="PSUM"))
        psu = ctx.enter_context(tc.tile_pool(name="psu", bufs=2, space="PSUM"))
        pst = ctx.enter_context(tc.tile_pool(name="pst", bufs=2, space="PSUM"))

        id16 = sb.tile([128, 128], F16, name="id16", tag="id16")
        make_identity(nc, id16[:])

        def load_split(d1, d2, n_tiles, width, prefix):
            t1, t2 = [], []
            for k in range(n_tiles):
                a = sb.tile([128, width], F16, name=f"{prefix}1_{k}", tag=f"{prefix}1_{k}")
                b = sb.tile([128, width], F16, name=f"{prefix}2_{k}", tag=f"{prefix}2_{k}")
                nc.sync.dma_start(a[:], d1.ap()[k * 128 : (k + 1) * 128, :])
                nc.sync.dma_start(b[:], d2.ap()[k * 128 : (k + 1) * 128, :])
                t1.append(a)
                t2.append(b)
            return t1, t2

        def load_w(li, n, k, suffix="", eng=None):
            eng = eng or nc.sync
            w1 = wp.tile([128, 512], F16, name=f"w1_{li}_{n}_{k}{suffix}", tag="w1")
            w2 = wp.tile([128, 512], F16, name=f"w2_{li}_{n}_{k}{suffix}", tag="w2")
            eng.dma_start(w1[:], A1[li].ap()[k * 128 : (k + 1) * 128, n * 512 : (n + 1) * 512])
            eng.dma_start(w2[:], A2[li].ap()[k * 128 : (k + 1) * 128, n * 512 : (n + 1) * 512])
            return w1, w2

        # first weight pairs prefetched ahead of the 16-tile xT bulk load so
        # the first matmul group isn't queued behind it (measured 11.7 us
        # startup PE stall otherwise)
        w_pre0 = {(0, k): load_w(0, 0, k, "p", eng=nc.scalar) for k in range(3)}
        xT_1, xT_2 = load_split(xT1, xT2, KD, BS, "xT")

        def mm3(p, l1, l2, r1, r2, first, last):
            nc.tensor.matmul(p, l1, r1, start=first, stop=False)
            nc.tensor.matmul(p, l2, r1, start=False, stop=False)
            nc.tensor.matmul(p, l1, r2, start=False, stop=last)

        def mm4(p, l1, l2, r1, r2, first, last):
            # 4-term variant: the dropped X2@Y2 cross term matters when the
            # downstream Hebbian reduction multiplies by a nonneg-mean
            # activation (no cancellation of fixed perturbations).
            nc.tensor.matmul(p, l1, r1, start=first, stop=False)
            nc.tensor.matmul(p, l2, r1, start=False, stop=False)
            nc.tensor.matmul(p, l1, r2, start=False, stop=False)
            nc.tensor.matmul(p, l2, r2, start=False, stop=last)

        def forward(li, L1, L2, evict, mm=None, w_pre=None):
            """evict(m, n, psum[128,512]) consumes each forward output tile."""
            mm = mm or mm3
            for n in range(NH):
                pt = [ps.tile([128, 512], F32, name=f"fw{li}_{m}_{n}", tag="ps") for m in range(MB)]
                for k in range(KD):
                    if w_pre and (n, k) in w_pre:
                        w1, w2 = w_pre[(n, k)]
                    else:
                        w1, w2 = load_w(li, n, k)
                    for m in range(MB):
                        msl = slice(m * 128, (m + 1) * 128)
                        mm(
                            pt[m][:],
                            L1[k][:, msl], L2[k][:, msl],
                            w1[:], w2[:],
                            first=(k == 0), last=(k == KD - 1),
                        )
                for m in range(MB):
                    evict(m, n, pt[m])

        def umatmul(ui, L1, L2, R1, R2, U_dram):
            """U partial = L.T @ R, contraction over the 512 batch rows."""
            for dm in range(KD):
                dsl = slice(dm * 128, (dm + 1) * 128)
                for n in range(NH):
                    nsl = slice(n * 512, (n + 1) * 512)
                    p = psu.tile([128, 512], F32, name=f"u{ui}_{dm}_{n}", tag="psu")
                    for k in range(MB):
                        mm3(
                            p[:],
                            L1[k][:, dsl], L2[k][:, dsl],
                            R1[k][:, nsl], R2[k][:, nsl],
                            first=(k == 0), last=(k == MB - 1),
                        )
                    t = uev.tile([128, 512], F32, name=f"uev{ui}_{dm}_{n}", tag="uev")
                    nc.vector.tensor_copy(t[:], p[:])
                    nc.sync.dma_start(U_dram.ap()[dsl, nsl], t[:])

        def transpose_forms(nat1, nat2, name):
            """Transpose fp16 form tiles [MB][128,H] -> [KD][128,BS]."""
            t1 = [sb.tile([128, BS], F16, name=f"{name}1_{d}", tag=f"{name}1_{d}") for d in range(KD)]
            t2 = [sb.tile([128, BS], F16, name=f"{name}2_{d}", tag=f"{name}2_{d}") for d in range(KD)]
            for m in range(MB):
                msl = slice(m * 128, (m + 1) * 128)
                for d in range(KD):
                    dsl = slice(d * 128, (d + 1) * 128)
                    p1 = pst.tile([128, 128], F16, name=f"tp1_{name}_{m}_{d}", tag="tp")
                    nc.tensor.transpose(p1[:], nat1[m][:, dsl], id16[:])
                    nc.vector.tensor_scalar_mul(t1[d][:, msl], p1[:], 16.0)
                    p2 = pst.tile([128, 128], F16, name=f"tp2_{name}_{m}_{d}", tag="tp")
                    nc.tensor.transpose(p2[:], nat2[m][:, dsl], id16[:])
                    nc.vector.tensor_scalar_mul(t2[d][:, msl], p2[:], 16.0)
            return t1, t2

        def hx_group(name):
            g1 = [hx.tile([128, H], F16, name=f"{name}1_{m}", tag=f"hx1_{m}") for m in range(MB)]
            g2 = [hx.tile([128, H], F16, name=f"{name}2_{m}", tag=f"hx2_{m}") for m in range(MB)]
            return g1, g2

        # ---- layer 1: h0 = relu(x @ A0) ----
        h0x1, h0x2 = hx_group("h0x")
        h0_1 = [sb.tile([128, H], F16, name=f"h0n1_{m}", tag=f"h0n1_{m}") for m in range(MB)]
        h0_2 = [sb.tile([128, H], F16, name=f"h0n2_{m}", tag=f"h0n2_{m}") for m in range(MB)]

        def ev_h0(m, n, p):
            nsl = slice(n * 512, (n + 1) * 512)
            q = scr.tile([128, 512], F32, name=f"q0_{m}_{n}", tag="scr")
            nc.vector.tensor_scalar_mul(q[:], p[:], 0.03125)
            nc.vector.tensor_copy(h0x1[m][:, nsl], q[:])
            nc.vector.tensor_tensor(h0x2[m][:, nsl], q[:], h0x1[m][:, nsl], SUB)
            # h0 kept pre-scaled by 1/16 so the U partial sums accumulate at
            # 16x smaller magnitude (less fp32 PSUM ulp noise); the host
            # multiplies U2/U4 back, and transpose eviction un-scales for
            # the forward path. Power of two, so exact.
            r = scr.tile([128, 512], F32, name=f"r0_{m}_{n}", tag="scr")
            nc.vector.tensor_scalar(r[:], q[:], 0.0, 0.0625, mybir.AluOpType.max, mybir.AluOpType.mult)
            nc.vector.tensor_copy(h0_1[m][:, nsl], r[:])
            nc.vector.tensor_tensor(h0_2[m][:, nsl], r[:], h0_1[m][:, nsl], SUB)

        forward(0, xT_1, xT_2, ev_h0, w_pre=w_pre0)
        xn_1, xn_2 = load_split(xn1, xn2, MB, D, "xn")
        umatmul(0, xn_1, xn_2, h0x1, h0x2, U[0])
        h0T_1, h0T_2 = transpose_forms(h0_1, h0_2, "h0T")

        # ---- layer 2: h1 = relu(x @ A1 + h0 @ A2) ----
        h1x1, h1x2 = hx_group("h1x")

        def ev_h1x(m, n, p):
            nsl = slice(n * 512, (n + 1) * 512)
            q = scr.tile([128, 512], F32, name=f"q1_{m}_{n}", tag="scr")
            nc.vector.tensor_scalar_mul(q[:], p[:], 0.03125)
            nc.vector.tensor_copy(h1x1[m][:, nsl], q[:])
            nc.vector.tensor_tensor(h1x2[m][:, nsl], q[:], h1x1[m][:, nsl], SUB)

        forward(1, xT_1, xT_2, ev_h1x)
        umatmul(1, xn_1, xn_2, h1x1, h1x2, U[1])

        h10_1, h10_2 = hx_group("h10")
        h1_1 = [sb.tile([128, H], F16, name=f"h1n1_{m}", tag=f"h1n1_{m}") for m in range(MB)]
        h1_2 = [sb.tile([128, H], F16, name=f"h1n2_{m}", tag=f"h1n2_{m}") for m in range(MB)]

        def ev_h10(m, n, p):
            nsl = slice(n * 512, (n + 1) * 512)
            q = scr.tile([128, 512], F32, name=f"qh_{m}_{n}", tag="scr")
            nc.vector.tensor_scalar_mul(q[:], p[:], 0.03125)
            nc.vector.tensor_copy(h10_1[m][:, nsl], q[:])
            nc.vector.tensor_tensor(h10_2[m][:, nsl], q[:], h10_1[m][:, nsl], SUB)
            r = scr.tile([128, 512], F32, name=f"r1_{m}_{n}", tag="scr")
            nc.vector.tensor_tensor(r[:], q[:], h1x1[m][:, nsl], ADD)
            nc.vector.tensor_tensor(r[:], r[:], h1x2[m][:, nsl], ADD)
            nc.vector.tensor_scalar(r[:], r[:], 0.0, 0.0625, mybir.AluOpType.max, mybir.AluOpType.mult)
            nc.vector.tensor_copy(h1_1[m][:, nsl], r[:])
            nc.vector.tensor_tensor(h1_2[m][:, nsl], r[:], h1_1[m][:, nsl], SUB)

        forward(2, h0T_1, h0T_2, ev_h10, mm=mm4)
        umatmul(2, h0_1, h0_2, h10_1, h10_2, U[2])
        h1T_1, h1T_2 = transpose_forms(h1_1, h1_2, "h1T")

        # ---- layer 3: y = x @ A3 + h0 @ A4 + h1 @ A5 (accum-DMA'd into y) ----
        def ev_y(pref, yt1, yt2):
            def _ev(m, n, p):
                nsl = slice(n * 512, (n + 1) * 512)
                t = uev.tile([128, 512], F32, name=f"yev_{pref}_{m}_{n}", tag="uev")
                nc.vector.tensor_scalar_mul(t[:], p[:], 0.03125)
                nc.vector.tensor_copy(yt1[m][:, nsl], t[:])
                nc.vector.tensor_tensor(yt2[m][:, nsl], t[:], yt1[m][:, nsl], SUB)
                nc.gpsimd.dma_start(
                    y_d.ap()[m * 128 : (m + 1) * 128, n * 512 : (n + 1) * 512],
                    t[:],
                    accum_op=ADD,
                )
            return _ev

        yx1, yx2 = hx_group("yx")
        forward(3, xT_1, xT_2, ev_y("yx", yx1, yx2))
        umatmul(3, xn_1, xn_2, yx1, yx2, U[3])

        y01, y02 = hx_group("y0")
        forward(4, h0T_1, h0T_2, ev_y("y0", y01, y02), mm=mm4)
        umatmul(4, h0_1, h0_2, y01, y02, U[4])

        y11, y12 = hx_group("y1")
        forward(5, h1T_1, h1T_2, ev_y("y1", y11, y12), mm=mm4)
        umatmul(5, h1_1, h1_2, y11, y12, U[5])

    nc.finalize()
    return nc


def _get_nc():
    if "nc" not in _CACHE:
        _CACHE["nc"] = _build()
    return _CACHE["nc"]


def _split(a):
    p = a.astype(np.float16)
    r = (a - p.astype(np.float32)).astype(np.float16)
    return p, r


def run_device(inputs, trace=False):
    x = np.ascontiguousarray(inputs["x"], dtype=np.float32)
    W = np.float32(inputs["W"].reshape(-1)[0])
    w_names = ["x2h0", "x2h1", "h02h1", "x2y", "h02y", "h12y"]
    A_split = [
        _split(
            np.float32(32.0)
            * (inputs[n].astype(np.float32) + W * inputs["heb_" + n].astype(np.float32))
        )
        for n in w_names
    ]
    xT_full = np.ascontiguousarray(x.T)

    in_maps = []
    for c in range(NCORES):
        xTp, xTr = _split(np.ascontiguousarray(xT_full[:, c * BS : (c + 1) * BS]))
        xnp_, xnr = _split(np.ascontiguousarray(x[c * BS : (c + 1) * BS, :]))
        m = {"xT1": xTp, "xT2": xTr, "xn1": xnp_, "xn2": xnr}
        for i in range(6):
            m[f"A{i}p"] = A_split[i][0]
            m[f"A{i}r"] = A_split[i][1]
        in_maps.append(m)

    nc = _get_nc()
    return bass_utils.run_bass_kernel_spmd(
        nc, in_maps, core_ids=list(range(NCORES)), trace=trace
    )


def kernel(**inputs):
    inputs = {k: np.asarray(v) for k, v in inputs.items()}
    res = run_device(inputs)
    results = res.results

    y = np.concatenate([results[c]["y"] for c in range(NCORES)], axis=0)

    w_names = ["x2h0", "x2h1", "h02h1", "x2y", "h02y", "h12y"]
    hebs_new = []
    for i, n in enumerate(w_names):
        U_tot = results[0][f"U{i}"].astype(np.float32).copy()
        for c in range(1, NCORES):
            U_tot += results[c][f"U{i}"]
        if i in (2, 4, 5):
            U_tot *= np.float32(16.0)  # undo the 1/16 pre-scale of h0/h1
        heb_n = np.clip(
            inputs["heb_" + n].astype(np.float32) + ALPHA * U_tot, -CLAMP, CLAMP
        ).astype(np.float32)
        hebs_new.append(heb_n)

    y_out = np.ascontiguousarray(y[:, : O - 2])
    W_new = np.float32(np.tanh(y[:, O - 2]).mean())
    dopa = np.float32(np.tanh(y[:, O - 3]).mean())

    return (y_out, *hebs_new, W_new, dopa)
